# revision 7
# baseline (speedup 1.0000x reference)
"""Trainium2 Bass kernel for ModalitySpecificLocalSelfAttention (7x7 window).

v3: col-major layout (see v2 docstring) with per-instruction overheads
minimized after hardware profiling:
  - conv epilogues run on PAIRS of PSUM banks ([C,1024] per op), ACT mostly;
  - softmax stats at GROUP granularity (4 blocks): one reduce (Z includes a
    pre-written oob column), one reciprocal, one stride-0-broadcast
    tensor_mul normalize;
  - all 32 per-block XBAR transposes collapsed to 4 strided gather DMAs +
    4 batched wavt XBARs (gpsimd-issued) and 4 batched a-XBARs (sync);
  - S/mask matmuls at width 308 (tails handled by one-time memsets).
"""

import sys

for _p in ("/opt/trn_rl_repo", "/root/.axon_site/_ro/trn_rl_repo"):
    if _p not in sys.path:
        sys.path.append(_p)

import ml_dtypes
import numpy as np

import concourse.bass as bass
from concourse import mybir
from concourse.bass_utils import run_bass_kernel_spmd

F32 = mybir.dt.float32
BF16 = mybir.dt.bfloat16

C = 128
H = 128
W = 128
NCORES = 8
RPC = H // NCORES          # 16 rows per core
PAD = 3
HR = RPC + 2 * PAD         # 22 halo rows
WPC = W + 2 * PAD          # 134 padded cols
RECT = HR * WPC            # 2948
RECTA = 3072               # allocated (24 x 128)
OWN = RPC * W              # 2048 owned pixels
NB = 16                    # blocks of 8 image cols x 16 rows
BPX = 128                  # pixels per block
NW = 384                   # padded neighborhood width (real 308)
NWR = 308                  # real neighborhood (14 cols x 22 rows)
WSTRIDE = 8 * HR           # 176
MR = 24                    # mask rank
CH = 512                   # conv chunk (one PSUM bank of f32)
ESH = -16.0
MBIG = -100.0

NKC = RECTA // CH          # 6 chunks (3 pairs) rect convs
NQC = OWN // CH            # 4 chunks (2 pairs) owned convs


def _build_program():
    nc = bass.Bass("TRN2", target_bir_lowering=False, debug=False)

    # ---- DRAM I/O ----
    xs_d = nc.dram_tensor("xs", [C, RECTA], BF16, kind="ExternalInput").ap()
    wall_d = nc.dram_tensor("wall", [C, 7 * C], BF16, kind="ExternalInput").ap()
    lr_d = nc.dram_tensor("lr", [MR, BPX + NWR], BF16, kind="ExternalInput").ap()
    sm_d = nc.dram_tensor("sm", [1, C + CH], BF16, kind="ExternalInput").ap()
    bias_d = nc.dram_tensor("bias", [C, 7], F32, kind="ExternalInput").ap()
    y_d = nc.dram_tensor("y", [C, OWN], F32, kind="ExternalOutput").ap()

    # ---- SBUF ----
    sb = lambda name, shape, dt: nc.alloc_sbuf_tensor(name, list(shape), dt).ap()
    xs = sb("xs_sb", [C, RECTA], BF16)
    k1 = sb("k1_sb", [C, RECTA], BF16)
    kpad = sb("kpad_sb", [C, RECTA], BF16)
    vpad = sb("vpad_sb", [C, RECTA], BF16)
    wav = sb("wav_sb", [C, RECTA], BF16)
    q1 = sb("q1_sb", [C, OWN], BF16)
    q = sb("q_sb", [C, OWN], BF16)
    wall = sb("wall_sb", [C, 7 * C], BF16)
    lr = sb("lr_sb", [MR, BPX + NWR], BF16)
    sm = sb("sm_sb", [1, C + CH], BF16)
    bias = sb("bias_sb", [C, 7], F32)
    ae = sb("ae_sb", [C, NB, NW], BF16)      # exp; col 308 = oob Z term
    aa = sb("aa_sb", [C, NB, NW], BF16)      # normalized attention
    at = sb("at_sb", [C, 3 * NB, C], BF16)
    wavt = sb("wavt_sb", [C, 3 * NB, C], BF16)
    wavg = sb("wavg_sb", [C, 2, 4, NW], BF16)  # gathered wav windows (parity)
    zz = sb("zz_sb", [C, 8], F32)
    zz16 = sb("zz16_sb", [C, 8], BF16)
    rz16 = sb("rz16_sb", [C, 8], BF16)
    yt = sb("yt_sb", [C, 2, CH], F32)
    esh = sb("esh_sb", [C, 1], F32)
    escr = sb("escr_sb", [C, 2 * CH], BF16)

    W_NAMES = ("wk1t", "wq1t", "wk2t", "wvt", "wq2t", "wxt", "wavw")
    w_sb = {n: wall[:, bass.ts(i, C)] for i, n in enumerate(W_NAMES)}
    Lm = lr[:, 0:BPX]
    Rm = lr[:, BPX:BPX + NWR]
    bo_row = sm[:, 0:C]
    ones_row = sm[:, C:C + CH]
    b_col = {n: bias[:, i:i + 1]
             for i, n in enumerate(("bk1", "bq1", "bk2", "bv", "bq2"))}
    oobc = bias[:, 5:6]
    bo_col = bias[:, 6:7]

    # ---- PSUM: 3 conv/S pair-tensors (6 banks) + 2 y banks ----
    ps_sp = [nc.alloc_psum_tensor(f"ps_s{i}", [C, 2, CH], F32).ap()
             for i in range(3)]
    ps_yp = [nc.alloc_psum_tensor(f"ps_y{i}", [C, CH], F32).ap()
             for i in range(2)]

    def s_bank(bc):
        return ps_sp[(bc // 2) % 3][:, bc % 2, :]

    # ---- semaphores ----
    sem_names = (["swall", "sdx0", "sdx1", "sdw", "sp", "sa", "sv", "sg",
                  "syd0", "syd1"]
                 + [f"sgw{j}" for j in range(4)]
                 + [f"swt{j}" for j in range(4)]
                 + [f"sax{j}" for j in range(4)])
    sems = {n: nc.alloc_semaphore(n) for n in sem_names}

    ENGS = ("sync", "pe", "act", "dve", "gp")
    plan = {e: [] for e in ENGS}
    cnt = {n: 0 for n in sem_names}

    def op(eng, fn, sem, inc=1):
        plan[eng].append(("op", fn, sem, inc))
        if sem:
            cnt[sem] += inc
            return (sem, cnt[sem])
        return None

    def wait(eng, mark):
        if mark:
            s_, v = mark
            if v > 0:
                plan[eng].append(("w", s_, v))

    RELU = mybir.ActivationFunctionType.Relu
    EXP = mybir.ActivationFunctionType.Exp
    ADD = mybir.AluOpType.add
    MAX = mybir.AluOpType.max

    # ================= input DMAs =================
    WALLM = op("sync", lambda: nc.sync.dma_start(
        out=wall, in_=wall_d), "swall", 16)
    XP0 = op("sync", lambda: nc.sync.dma_start(
        out=xs[:, 0:1536], in_=xs_d[:, 0:1536]), "sdx0", 16)
    XP1 = op("sync", lambda: nc.sync.dma_start(
        out=xs[:, 1536:RECTA], in_=xs_d[:, 1536:RECTA]), "sdx1", 16)
    for dst, srcd in ((lr, lr_d), (sm, sm_d), (bias, bias_d)):
        SDW = op("sync", lambda d=dst, s=srcd:
                 nc.sync.dma_start(out=d, in_=s), "sdw", 16)

    ESHM = op("dve", lambda: nc.vector.memset(esh, ESH), "sv")
    wait("act", ESHM)
    op("act", lambda: nc.scalar.activation(zz[:, 0:1], esh, RELU), "sa")

    # one-time: ae oob column + aa tail zeros (gpsimd, SBUF only)
    oob_bc = bass.AP(tensor=bias.tensor, offset=oobc.offset,
                     ap=[[7, C], [0, NB], [1, 1]])
    wait("gp", SDW)
    AEOOB = op("gp", lambda: nc.gpsimd.tensor_copy(
        ae[:, :, NWR:NWR + 1], oob_bc), "sg")
    AATAIL = op("gp", lambda: nc.gpsimd.memset(aa[:, :, NWR:NW], 0.0), "sg")

    # ---- PE warmup: ramp the clock while inputs stream in ----
    wait("pe", WALLM)
    for _w in range(6):
        op("pe", lambda: nc.tensor.matmul(
            ps_sp[0][:, 0, :], w_sb["wk1t"], wall[:, 0:CH],
            start=True, stop=True), "sp")

    # ================= convs (pair granularity) =================
    xsr = xs[:, 0:RECT].rearrange("p (c r) -> p c r", r=HR)

    def rect_rhs(src):
        return lambda j: src[:, bass.ts(j, CH)]

    def q1_rhs(j):
        return xsr[:, PAD + 32 * j:PAD + 32 * (j + 1), PAD:PAD + RPC]

    conv_list = {
        "k1": ("wk1t", rect_rhs(xs), k1, "bk1"),
        "q1": ("wq1t", q1_rhs, q1, "bq1"),
        "k2": ("wk2t", rect_rhs(k1), kpad, "bk2"),
        "v": ("wvt", rect_rhs(xs), vpad, "bv"),
        "q2": ("wq2t", rect_rhs(q1), q, "bq2"),
        "wav": ("wavw", rect_rhs(vpad), wav, None),
    }
    src_of = {"k2": "k1", "q2": "q1", "wav": "v"}

    epi_done = {}        # (cname, pair) -> mark
    last_escr = [None]
    last_pt_user = {}    # pair-tensor idx -> mark
    pt_i = [0]
    mems_k, mems_v = [], []

    # epilogue engine per (conv, pair): ACT except two DVE split-op pairs
    DVE_EPIS = {("k1", 1), ("k2", 1), ("v", 1), ("q1", 1), ("q2", 1)}

    def emit_wav_chunk(j):
        wn, rhsf, dst, bn = conv_list["wav"]
        ps = ps_yp[j % 2]
        if j == 0:
            for mk_ in mems_v:
                wait("pe", mk_)
        wait("pe", epi_done.get(("wavc", j - 2)))
        mm = op("pe", lambda p=ps, w_=w_sb[wn], r=rhsf(j):
                nc.tensor.matmul(p, w_, r, start=True, stop=True), "sp")
        wait("dve", mm)
        mk = op("dve", lambda o=dst[:, bass.ts(j, CH)], p=ps:
                nc.vector.tensor_copy(o, p), "sv")
        epi_done[("wavc", j)] = mk
        if j in (4, 5):
            epi_done[("wav", j - 3)] = mk
        elif j == 1:
            epi_done[("wav", 0)] = mk

    def emit_conv_pair(cname, s):
        wn, rhsf, dst, bn = conv_list[cname]
        pt = pt_i[0] % 3
        pt_i[0] += 1
        ps2 = ps_sp[pt]
        if cname in src_of:
            wait("pe", epi_done.get((src_of[cname], s)))
        wait("pe", last_pt_user.get(pt))
        if cname == "k1" or cname == "q1":
            if (cname, s) in (("k1", 0), ("q1", 0)):
                wait("pe", XP0)
                wait("pe", WALLM)
            else:
                wait("pe", XP1)
        mm = None
        for h in (0, 1):
            j = 2 * s + h
            mm = op("pe", lambda p=ps2[:, h, :], w_=w_sb[wn], r=rhsf(j):
                    nc.tensor.matmul(p, w_, r, start=True, stop=True), "sp")
        dpair = dst[:, 1024 * s:1024 * (s + 1)].rearrange(
            "p (a b) -> p a b", b=CH)
        if bn is None:
            wait("dve", SDW)
            wait("dve", mm)
            mk = op("dve", lambda o=dpair, p=ps2:
                    nc.vector.tensor_copy(o, p), "sv")
        elif (cname, s) in DVE_EPIS:
            wait("dve", SDW)
            wait("dve", mm)
            wait("dve", last_escr[0])
            cpm = op("dve", lambda o=escr, p=ps2:
                     nc.vector.tensor_copy(
                         o.rearrange("p (a b) -> p a b", b=CH), p), "sv")
            wait("dve", cpm)
            mk = op("dve", lambda o=dst[:, 1024 * s:1024 * (s + 1)], i_=escr,
                    b=b_col[bn]:
                    nc.vector.tensor_scalar(o, i_, b, 0.0, ADD, MAX), "sv")
            last_escr[0] = mk
        else:
            wait("act", SDW)
            wait("act", mm)
            mk = op("act", lambda o=dpair, p=ps2, b=b_col[bn]:
                    nc.scalar.activation(o, p, RELU, bias=b), "sa")
        epi_done[(cname, s)] = mk
        last_pt_user[pt] = mk

    for s in range(3):
        emit_conv_pair("k1", s)
        if s < 2:
            emit_conv_pair("q1", s)
    for s in range(3):
        emit_conv_pair("k2", s)
        emit_conv_pair("v", s)

    # pad-col memsets after the epilogues that wrote garbage there
    kpr = kpad[:, 0:RECT].rearrange("p (c r) -> p c r", r=HR)
    vpr = vpad[:, 0:RECT].rearrange("p (c r) -> p c r", r=HR)
    for t, lst, nm in ((kpr, mems_k, "k2"), (vpr, mems_v, "v")):
        wait("gp", epi_done[(nm, 0)])
        lst.append(op("gp", lambda tf=t[:, 0:PAD, :]:
                      nc.gpsimd.memset(tf, 0.0), "sg"))
        wait("gp", epi_done[(nm, 2)])
        lst.append(op("gp", lambda tf=t[:, PAD + W:WPC, :]:
                      nc.gpsimd.memset(tf, 0.0), "sg"))

    for s in range(2):
        emit_conv_pair("q2", s)

    WPAIR_HI = [0, 1, 2, 2]   # wav pair covering group g's windows
    gw_mark, wvt_mark = {}, {}

    def st_gather(g):
        wait("gp", epi_done[("wav", WPAIR_HI[g])])
        if g >= 2:
            wait("gp", (f"swt{g - 2}", 16))
        base = wav[:, WSTRIDE * 4 * g:WSTRIDE * 4 * g + NW]
        win = bass.AP(tensor=wav.tensor, offset=base.offset,
                      ap=[[RECTA, C], [WSTRIDE, 4], [1, NW]])
        gw_mark[g] = op("gp", lambda o=wavg[:, g % 2], i_=win:
                        nc.gpsimd.dma_start(out=o, in_=i_), f"sgw{g}", 16)
        wait("sync", gw_mark[g])
        wvt_mark[g] = op("sync", lambda o=wavt[:, 12 * g:12 * (g + 1), :],
                         i_=wavg[:, g % 2]:
                         nc.sync.dma_start(out=o, in_=i_, transpose=True),
                         f"swt{g}", 16)

    # ================= attention =================
    s_done, exp_done, norm_done, ax_mark, grp_done = {}, {}, {}, {}, {}
    ydma, ycopy = {}, {}

    def st_s(bc):
        ps = s_bank(bc)[:, 0:NWR]
        wait("pe", epi_done[("q2", bc // 8)])
        wait("pe", epi_done[("k2", min((WSTRIDE * bc + NWR - 1) // 1024, 2))])
        if bc == 0:
            for mk_ in mems_k:
                wait("pe", mk_)
            for pt_ in range(3):
                wait("pe", last_pt_user.get(pt_))
        if bc >= 6:
            wait("pe", exp_done[(bc - 6) // 2])
        op("pe", lambda o=ps, l=q[:, bass.ts(bc, BPX)],
                 r=kpad[:, bass.ds(WSTRIDE * bc, NWR)]:
           nc.tensor.matmul(o, l, r, start=True, stop=False), "sp")
        s_done[bc] = op("pe", lambda o=ps, l=Lm, r=Rm:
                        nc.tensor.matmul(o, l, r, start=False, stop=True),
                        "sp")

    def st_exp(i):  # pair i: blocks 2i, 2i+1
        bc = 2 * i
        wait("act", s_done[bc + 1])
        wait("act", ESHM)
        exp_done[i] = op(
            "act",
            lambda o=ae[:, bc:bc + 2, 0:NWR], i_=ps_sp[i % 3][:, :, 0:NWR]:
                nc.scalar.activation(o, i_, EXP, bias=esh), "sa")

    def st_softmax_group(g):  # blocks 4g..4g+3
        c0 = 4 * (g % 2)
        wait("dve", exp_done[2 * g + 1])
        wait("dve", AEOOB)
        def _zred(o=zz16[:, c0:c0 + 4], i_=ae[:, 4 * g:4 * g + 4, 0:NWR + 1]):
            with nc.allow_low_precision(reason="softmax Z in bf16"):
                return nc.vector.reduce_sum(o, i_, axis=mybir.AxisListType.X)
        zr = op("dve", _zred, "sv")
        wait("dve", zr)
        def _recip(o=rz16[:, c0:c0 + 4], i_=zz16[:, c0:c0 + 4]):
            with nc.allow_low_precision(reason="softmax 1/Z in bf16"):
                return nc.vector.reciprocal(o, i_)
        rc = op("dve", _recip, "sv")
        wait("dve", rc)
        rzb = bass.AP(tensor=rz16.tensor, offset=rz16[:, c0:c0 + 4].offset,
                      ap=[[8, C], [1, 4], [0, NWR]])
        norm_done[g] = op(
            "dve",
            lambda o=aa[:, 4 * g:4 * g + 4, 0:NWR],
                   i_=ae[:, 4 * g:4 * g + 4, 0:NWR], r=rzb:
                nc.vector.tensor_mul(o, i_, r), "sv")

    def st_ax(g):
        wait("sync", norm_done[g])
        wait("sync", AATAIL)
        ax_mark[g] = op(
            "sync",
            lambda o=at[:, 12 * g:12 * (g + 1), :], i_=aa[:, 4 * g:4 * g + 4, :]:
                nc.sync.dma_start(out=o, in_=i_, transpose=True),
            f"sax{g}", 16)

    def st_group(g):
        pq = g % 2
        ps = ps_yp[pq]
        wait("pe", epi_done[("wavc", 4 + pq)])
        if g >= 2:
            wait("pe", ycopy[g - 2])
        op("pe", lambda o=ps, l=w_sb["wxt"],
                 r=xsr[:, PAD + 32 * g:PAD + 32 * (g + 1), PAD:PAD + RPC]:
           nc.tensor.matmul(o, l, r, start=True, stop=False,
                            skip_group_check=True), "sp")
        wait("pe", (f"sax{g}", 16))
        wait("pe", (f"swt{g}", 16))
        last = None
        for i in range(4):
            bc = 4 * g + i
            for t in range(3):
                fin = (i == 3 and t == 2)
                last = op(
                    "pe",
                    lambda o=ps[:, bass.ts(i, BPX)],
                           l=wavt[:, 3 * bc + t, :], r=at[:, 3 * bc + t, :],
                           sp_=fin:
                        nc.tensor.matmul(o, l, r, start=False, stop=sp_,
                                         skip_group_check=True), "sp")
        grp_done[g] = last

    for bc in range(NB):
        st_s(bc)
        if bc == 3:
            for j in range(NKC):
                emit_wav_chunk(j)
        if bc % 2 == 1:
            st_exp(bc // 2)
        if bc % 4 == 3:
            st_gather(bc // 4)
            st_softmax_group(bc // 4)
            st_ax(bc // 4)

    for g in range(4):
        st_group(g)
        ceng = ("act", "dve")[g % 2]
        wait(ceng, grp_done[g])
        if g >= 2:
            wait(ceng, ydma[g - 2])
        if ceng == "act":
            ycopy[g] = op("act", lambda o=yt[:, g % 2, :], i_=ps_yp[g % 2]:
                          nc.scalar.activation(
                              o, i_, mybir.ActivationFunctionType.Identity,
                              bias=bo_col), "sa")
        else:
            ycopy[g] = op("dve", lambda o=yt[:, g % 2, :], i_=ps_yp[g % 2]:
                          nc.vector.tensor_scalar_add(o, i_, bo_col), "sv")
        wait("gp", ycopy[g])
        ydma[g] = op(
            "gp",
            lambda o=y_d[:, bass.ts(g, CH)], i_=yt[:, g % 2, :]:
                nc.gpsimd.dma_start(out=o, in_=i_),
            f"syd{g % 2}", 16)

    # ---- tail barrier ----
    for s_ in ("sp", "sa", "sv", "sg"):
        wait("sync", (s_, cnt[s_]))
    for s_ in ("sdx0", "sdx1", "sdw", "syd0", "syd1"):
        wait("sync", (s_, cnt[s_]))
    for j in range(4):
        for p_ in ("sgw", "swt", "sax"):
            wait("sync", (f"{p_}{j}", cnt[f"{p_}{j}"]))

    # ---- emit ----
    def run(eng_name, eng_obj):
        hwm = {}
        for item in plan[eng_name]:
            if item[0] == "w":
                _, s_, v = item
                if hwm.get(s_, 0) >= v:
                    continue
                hwm[s_] = v
                eng_obj.wait_ge(sems[s_], v)
            else:
                _, fn, s_, inc = item
                inst = fn()
                if s_:
                    inst.then_inc(sems[s_], inc)

    with nc.Block() as block:
        @block.sync
        def _(e):
            run("sync", e)

        @block.tensor
        def _(e):
            run("pe", e)

        @block.scalar
        def _(e):
            run("act", e)

        @block.vector
        def _(e):
            run("dve", e)

        @block.gpsimd
        def _(e):
            run("gp", e)

    with nc.Block() as block2:
        @block2.sync
        def _(e):
            for n in sem_names:
                nc.sync.sem_clear(sems[n])

    return nc


_PROGRAM = None


def _host_inputs(x, w_q1, s_q1, b_q1, w_q2, s_q2, b_q2,
                 w_k1, s_k1, b_k1, w_k2, s_k2, b_k2,
                 w_v, s_v, b_v, w_o, s_o, b_o):
    def foldT(w, s):
        return np.ascontiguousarray((s[:, None] * w).T.astype(ml_dtypes.bfloat16))

    wq1t, wq2t = foldT(w_q1, s_q1), foldT(w_q2, s_q2)
    wk1t, wk2t = foldT(w_k1, s_k1), foldT(w_k2, s_k2)
    wvt = foldT(w_v, s_v)
    wo = s_o[:, None] * np.asarray(w_o, np.float32)
    wat = np.ascontiguousarray(wo[:, :C].T.astype(ml_dtypes.bfloat16))
    wxt = np.ascontiguousarray(wo[:, C:].T.astype(ml_dtypes.bfloat16))
    wall = np.concatenate([wk1t, wq1t, wk2t, wvt, wq2t, wxt, wat], axis=1)

    L = np.zeros((MR, BPX), np.float32)
    for p in range(BPX):
        pc, pr = p // RPC, p % RPC
        L[pc, p] = 1.0
        L[8 + pr, p] = 1.0
    e16 = np.float32(np.exp(ESH))

    sm = np.zeros((1, C + CH), np.float32)
    sm[0, :C] = b_o
    sm[0, C:] = 1.0

    X = np.asarray(x, np.float32).reshape(C, H, W)
    shared = dict(
        wall=np.ascontiguousarray(wall),
        sm=np.ascontiguousarray(sm.astype(ml_dtypes.bfloat16)),
    )

    col = lambda b: b.astype(np.float32)[:, None]
    in_maps = []
    for core in range(NCORES):
        h0 = core * RPC
        rect = np.zeros((C, HR, WPC), np.float32)
        lo, hi = h0 - PAD, h0 + RPC + PAD
        slo, shi = max(lo, 0), min(hi, H)
        rect[:, slo - lo:shi - lo, PAD:PAD + W] = X[:, slo:shi, :]
        xs_cm = np.zeros((C, RECTA), np.float32)
        xs_cm[:, :RECT] = rect.transpose(0, 2, 1).reshape(C, RECT)

        rowok = np.array([0 <= h0 + nr - PAD < H for nr in range(HR)])
        R = np.zeros((MR, NWR), np.float32)
        for n in range(NWR):
            ncol, nrow = n // HR, n % HR
            for k in range(8):
                if not (k <= ncol <= k + 6):
                    R[k, n] = MBIG
            for j in range(RPC):
                if not (j <= nrow <= j + 6 and rowok[nrow]):
                    R[8 + j, n] = MBIG
        lrm = np.concatenate([L, R], axis=1)

        oob = np.zeros((C, 1), np.float32)
        for p in range(BPX):
            pr = p % RPC
            n_oob = sum(1 for i in range(7) if not (0 <= h0 + pr - PAD + i < H))
            oob[p, 0] = 7 * n_oob * e16

        biases = np.concatenate(
            [col(b_k1), col(b_q1), col(b_k2), col(b_v), col(b_q2), oob,
             col(b_o)], axis=1)

        m = dict(shared)
        m["xs"] = np.ascontiguousarray(xs_cm.astype(ml_dtypes.bfloat16))
        m["lr"] = np.ascontiguousarray(lrm.astype(ml_dtypes.bfloat16))
        m["bias"] = np.ascontiguousarray(biases.astype(np.float32))
        in_maps.append(m)
    return in_maps


def kernel(**inputs):
    global _PROGRAM
    if _PROGRAM is None:
        _PROGRAM = _build_program()
    in_maps = _host_inputs(**{k: np.asarray(v) for k, v in inputs.items()})
    res = run_bass_kernel_spmd(_PROGRAM, in_maps, core_ids=list(range(NCORES)))
    stripes = []
    for r in res.results:
        y = np.asarray(r["y"]).reshape(C, W, RPC)
        stripes.append(y.transpose(0, 2, 1))
    return np.concatenate(stripes, axis=1).reshape(1, C, H, W)


if __name__ == "__main__":
    rng = np.random.default_rng(0)
    fake = {"x": rng.standard_normal((1, C, H, W), np.float32)}
    for n in ("q1", "q2", "k1", "k2", "v", "o"):
        cin = 2 * C if n == "o" else C
        fake["w_" + n] = (rng.standard_normal((C, cin)) / np.sqrt(cin)).astype(np.float32)
        fake["s_" + n] = rng.uniform(0.5, 1.5, C).astype(np.float32)
        fake["b_" + n] = (rng.standard_normal(C) * 0.1).astype(np.float32)
    out = kernel(**fake)
    print("kernel output", out.shape, out.dtype)
